# revision 34
# baseline (speedup 1.0000x reference)
"""Trainium2 Bass kernel: FiLM-conditioned 1x1-conv self-attention block.

Sharding: 8 cores = 2 batches x 4 heads. Each core computes one (batch, head)
pair end-to-end, producing a partial output projection [C, N]; the host sums
the 4 head partials per batch (b_out is added on the h==0 cores only).

Math notes (all exact re-associations of the reference):
  - FiLM: x~ = x*(1+scale) + shift  =>  W @ x~ = (W*diag(1+scale)) @ x + (W@shift)
    so the modulation is folded into the QKV weight columns + a rank-1 bias.
  - silu(t) = t * 1/(1+exp(-t)) is computed with the EXP table only (keeps the
    scalar engine on one activation table set for the whole kernel).
  - The k-side FiLM bias is dropped: softmax over j is invariant to terms
    constant in j, so only q needs its rank-1 bias.
  - Attention is computed in transposed layout S^T[j,i] = sum_d k[d,j] q[d,i];
    softmax scale 1/sqrt(d) is folded into the exp activation's free scale.
    No max-subtraction (scores are O(5), exp is safe in fp32).
  - U = [V; 1]^T @ P^T accumulated over j gives both the unnormalized output
    (rows 0..32) and the softmax denominator (row 32) in one matmul chain.

Performance structure (ACT-engine bound: n^2 = 16.8M exp elems/core ~ 109us):
  - scores for 4 j-tiles land in ONE [128, 2048] bf16 PSUM tile -> a single
    wide ACTIVATE per quad (amortizes the ~293ns/instr ACT overhead).
  - S and U PSUM double-buffered so the exp stream never waits on the
    normalization chain of the previous i-tile.
  - prologue: all DMAs issued immediately, x converted in chunks overlapped
    with the FiLM weight-fold chain; first exp at ~10us instead of ~50us.
"""

import functools
import sys

import numpy as np

if "/opt/trn_rl_repo" not in sys.path:
    sys.path.insert(0, "/opt/trn_rl_repo")

HEADS = 4
D = 32              # dim head
C = 256             # channels
TD = 512            # time embedding dim
SCALE = D ** -0.5
N_FULL = 4096       # 64*64 spatial positions
NT = 512            # query (i) tile
JT = 128            # key (j) tile
QUAD = 4 * NT       # 4 j-tiles' scores per PSUM tile / exp
N_CORES = 8


def _build_program(n_pos=N_FULL):
    import concourse.bass as bass
    import concourse.tile as tile
    from concourse import bacc, mybir
    from concourse.masks import make_identity

    f32 = mybir.dt.float32
    bf16 = mybir.dt.bfloat16
    AF = mybir.ActivationFunctionType

    nc = bacc.Bacc("TRN2", debug=False)

    xb = nc.dram_tensor("xb", [C, n_pos], f32, kind="ExternalInput").ap()
    te = nc.dram_tensor("te", [TD], f32, kind="ExternalInput").ap()
    w_mlp = nc.dram_tensor("w_mlp", [TD, TD], f32, kind="ExternalInput").ap()
    b_mlp = nc.dram_tensor("b_mlp", [TD], f32, kind="ExternalInput").ap()
    wqkv = nc.dram_tensor("wqkv", [3 * D, C], f32, kind="ExternalInput").ap()
    wo = nc.dram_tensor("wo", [C, D], f32, kind="ExternalInput").ap()
    bo = nc.dram_tensor("bo", [C], f32, kind="ExternalInput").ap()
    out = nc.dram_tensor("out", [C, n_pos], f32, kind="ExternalOutput").ap()

    n_itiles = n_pos // NT
    n_jtiles = n_pos // JT
    n_quads = n_jtiles // 4
    XCHUNK = min(1024, n_pos)
    n_xchunks = n_pos // XCHUNK

    with tile.TileContext(nc) as tc:
        with tc.tile_pool(name="const", bufs=1) as const, \
             tc.tile_pool(name="xio", bufs=4) as xio:
            ident = const.tile([128, 128], f32, name="ident")
            make_identity(nc, ident)
            ident_bf = const.tile([128, 128], bf16, name="ident_bf")
            make_identity(nc, ident_bf)

            # persistent big tiles
            x_sb = [const.tile([128, n_pos], bf16, name=f"x_sb{cc}")
                    for cc in range(2)]
            q4 = const.tile([128, n_pos], bf16, name="q4")
            k4 = const.tile([128, n_pos], bf16, name="k4")
            # V with a ones row appended (row D, memset once): the PE
            # transpose then emits [128, D+1] V^T tiles with the ones column
            # built in, so vt copies are plain contiguous 2D copies.
            # rows: 0..D-1 = v, D = ones, D+1 = zero pad (so the PE
            # transpose writes full 34-col blocks at 4B-aligned offsets)
            v_sb = const.tile([D + 2, n_pos], bf16, name="v_sb")
            VTS = D + 2  # vt block stride
            vt_all = const.tile([128, n_jtiles * VTS], bf16, name="vt_all")

            # FiLM results
            tfull = const.tile([128, 4], f32, name="tfull")
            sc1 = const.tile([128, 2], f32, name="sc1")
            # folded qkv weights (transposed, q/k replicated 4x along M)
            q4T = [const.tile([128, 128], bf16, name=f"q4T{cc}") for cc in range(2)]
            k4T = [const.tile([128, 128], bf16, name=f"k4T{cc}") for cc in range(2)]
            vT_w = [const.tile([128, D], bf16, name=f"vT_w{cc}") for cc in range(2)]
            bq4 = const.tile([128, 1], f32, name="bq4")
            bv = const.tile([D, 1], f32, name="bv")
            # proj weights with a bo row appended: row D carries b_out so the
            # (deferred) normalization multiply turns bo*denom into bo exactly
            woT_aug = [const.tile([D + 1, 128], bf16, name=f"woT{hh}")
                       for hh in range(2)]

            # x fp32 staging chunks (DMA'd up front, converted lazily)
            x_stage = [[xio.tile([128, XCHUNK], f32, name=f"x_st_{cc}_{k}",
                                 tag=f"x_st_{cc}_{k}", bufs=1)
                        for k in range(n_xchunks)] for cc in range(2)]

            # -------- DMAs: everything in flight immediately --------
            # Each dma_start costs ~0.7us of serial dispatch on its queue
            # engine, so: few wide DMAs, weights on sync, x chunks dispatched
            # from the (otherwise idle) ACT hardware DMA queue.
            te_t = const.tile([128, 4], f32, name="te_t")
            nc.sync.dma_start(out=te_t, in_=te.rearrange("(f p) -> p f", p=128))
            # w_mlp: wide [128, 512] slabs -> efficient descriptors, 4
            # parallel queues; gates the FiLM -> weight-fold -> qkv chain.
            wm_slab = []
            for ot in range(4):
                t_ = xio.tile([128, TD], f32, name=f"wm_slab_{ot}",
                              tag=f"wm_slab_{ot}", bufs=1)
                nc.sync.dma_start(out=t_, in_=w_mlp[ot * 128:(ot + 1) * 128, :])
                wm_slab.append(t_)
            bm_t = const.tile([128, 4], f32, name="bm_t")
            nc.sync.dma_start(out=bm_t, in_=b_mlp.rearrange("(f p) -> p f", p=128))
            # first x chunks on sync right after w_mlp: converted on the
            # idle ACT engine before the exp stream starts.
            n_xsync = min(2, n_xchunks)
            for k in range(n_xsync):
                for cc in range(2):
                    nc.sync.dma_start(
                        out=x_stage[cc][k],
                        in_=xb[128 * cc:128 * (cc + 1),
                               k * XCHUNK:(k + 1) * XCHUNK])
            wqkv_nat = const.tile([D, 3 * C], f32, name="wqkv_nat")
            nc.sync.dma_start(
                out=wqkv_nat.rearrange("d (g c) -> d g c", c=C),
                in_=wqkv.rearrange("(g d) c -> d g c", d=D))
            w_nat = {"q": wqkv_nat[:, 0:C], "k": wqkv_nat[:, C:2 * C],
                     "v": wqkv_nat[:, 2 * C:3 * C]}
            wo_both = const.tile([128, 2 * D], f32, name="wo_both")
            nc.sync.dma_start(
                out=wo_both.rearrange("p (h d) -> p h d", d=D),
                in_=wo.rearrange("(h p) d -> p h d", p=128))
            wo_nat = [wo_both[:, 0:D], wo_both[:, D:2 * D]]
            bo_dma = const.tile([128, 2], f32, name="bo_dma")
            nc.sync.dma_start(out=bo_dma, in_=bo.rearrange("(f p) -> p f", p=128))

            # ones + pad rows of V (written once, before any v tile copy;
            # partition slices must start at multiples of 32, so set both
            # rows to 1.0 in one op -- the pad column is never read back)
            nc.vector.memset(v_sb[D:D + 2, :], 1.0)

            # ---------------- prologue: FiLM + weight prep ----------------
            with tc.tile_pool(name="pro_sb", bufs=3) as pro_sb, \
                 tc.tile_pool(name="pro_ps", bufs=2, space="PSUM") as pro_ps:

                # silu(te) via the EXP table only: s = te / (1 + exp(-te))
                emt = pro_sb.tile([128, 4], f32, tag="emt")
                nc.scalar.activation(emt, te_t, AF.Exp, scale=-1.0)
                nc.vector.tensor_scalar_add(emt, emt, 1.0)
                rec = pro_sb.tile([128, 4], f32, tag="rec")
                nc.vector.reciprocal(rec, emt)
                s_t = const.tile([128, 4], f32, name="s_t")
                nc.vector.tensor_mul(s_t, te_t, rec)

                # remaining x chunks: dispatched on the ACT queue after the
                # table load + silu so they don't delay the first activation
                for k in range(n_xsync, n_xchunks):
                    for cc in range(2):
                        nc.scalar.dma_start(
                            out=x_stage[cc][k],
                            in_=xb[128 * cc:128 * (cc + 1),
                                   k * XCHUNK:(k + 1) * XCHUNK])

                s_bf = pro_sb.tile([128, 4], bf16, tag="s_bf")
                nc.vector.tensor_copy(s_bf, s_t)

                # W_mlp^T tiles via bf16 PE transpose (fp32 transposes need
                # double LDWEIGHTS passes; bf16 halves the PE time here)
                wm_bf = []
                for ot in range(4):
                    wb = pro_sb.tile([128, TD], bf16, tag=f"wm_bf_{ot}",
                                     name=f"wm_bf_{ot}", bufs=1)
                    nc.vector.tensor_copy(wb, wm_slab[ot])
                    wm_bf.append(wb)
                wmT = [[None] * 4 for _ in range(4)]
                for cc4 in range(4):
                    for ot in range(4):
                        ps_t = pro_ps.tile([128, 128], bf16, tag="tp", bufs=3)
                        nc.tensor.transpose(
                            ps_t,
                            wm_bf[ot][:, cc4 * 128:(cc4 + 1) * 128], ident_bf)
                        wmT_t = pro_sb.tile([128, 128], bf16,
                                            tag=f"wmT_{cc4}_{ot}",
                                            name=f"wmT_{cc4}_{ot}", bufs=1)
                        nc.vector.tensor_copy(wmT_t, ps_t)
                        wmT[cc4][ot] = wmT_t

                # t = W_mlp @ silu(te) + b_mlp
                for ot in range(4):
                    t_ps = pro_ps.tile([128, 1], f32, tag="t_ps")
                    for cc4 in range(4):
                        nc.tensor.matmul(
                            t_ps, wmT[cc4][ot], s_bf[:, cc4:cc4 + 1],
                            start=(cc4 == 0), stop=(cc4 == 3),
                        )
                    nc.vector.tensor_add(
                        tfull[:, ot:ot + 1], t_ps, bm_t[:, ot:ot + 1]
                    )
                # scale+1 for c-chunks (cols 0,1); shift is cols 2,3
                nc.vector.tensor_scalar_add(sc1, tfull[:, 0:2], 1.0)
                tf_bf = pro_sb.tile([128, 2], bf16, tag="tf_bf")
                nc.vector.tensor_copy(tf_bf, tfull[:, 2:4])

                # W_qkv head slices -> transpose -> scale cols by (1+scale)
                # (k keeps no bias: softmax is invariant to j-constant terms)
                wqkv_bf = pro_sb.tile([D, 3 * C], bf16, tag="wqkv_bf")
                nc.vector.tensor_copy(wqkv_bf, wqkv_nat)
                w_bf = {"q": wqkv_bf[:, 0:C], "k": wqkv_bf[:, C:2 * C],
                        "v": wqkv_bf[:, 2 * C:3 * C]}
                for name, dstT, nrep in (
                    ("q", q4T, 4), ("k", k4T, 4), ("v", vT_w, 1),
                ):
                    uT = [None, None]
                    for cc in range(2):
                        ps_t = pro_ps.tile([128, D], bf16, tag="tp", bufs=3)
                        nc.tensor.transpose(
                            ps_t, w_bf[name][:, 128 * cc:128 * (cc + 1)],
                            ident_bf[0:D, 0:D],
                        )
                        if name != "k":
                            uT_t = pro_sb.tile([128, D], bf16,
                                               tag=f"uT_{name}{cc}",
                                               name=f"uT_{name}{cc}", bufs=1)
                            nc.vector.tensor_copy(uT_t, ps_t)
                            uT[cc] = uT_t
                        nc.vector.tensor_scalar_mul(
                            dstT[cc][:, 0:D], ps_t, sc1[:, cc:cc + 1])
                        if nrep == 4:
                            nc.vector.tensor_copy(
                                dstT[cc][:, D:2 * D], dstT[cc][:, 0:D])
                            nc.vector.tensor_copy(
                                dstT[cc][:, 2 * D:4 * D], dstT[cc][:, 0:2 * D])
                    if name == "k":
                        continue
                    # bias_g = W_g @ shift (unscaled weights)
                    b_ps = pro_ps.tile([D, 1], f32, tag="b_ps")
                    for cc in range(2):
                        nc.tensor.matmul(
                            b_ps, uT[cc], tf_bf[:, cc:cc + 1],
                            start=(cc == 0), stop=(cc == 1),
                        )
                    if name == "q":
                        for r in range(4):
                            nc.vector.tensor_copy(bq4[D * r:D * (r + 1), :], b_ps)
                    else:
                        nc.vector.tensor_copy(bv, b_ps)

                # w_out^T halves with bo appended as row D
                wo_bf = pro_sb.tile([128, 2 * D], bf16, tag="wo_bf")
                nc.vector.tensor_copy(wo_bf, wo_both)
                bo_bf = pro_sb.tile([128, 2], bf16, tag="bo_bf")
                nc.vector.tensor_copy(bo_bf, bo_dma)
                for hh in range(2):
                    ps_t = pro_ps.tile([D + 1, 128], bf16, tag="tp", bufs=3)
                    nc.tensor.transpose(ps_t[0:D, :],
                                        wo_bf[:, D * hh:D * (hh + 1)], ident_bf)
                    nc.tensor.transpose(ps_t[D:D + 1, :],
                                        bo_bf[:, hh:hh + 1], ident_bf,
                                        tile_position=(0, 32))
                    nc.vector.tensor_copy(woT_aug[hh], ps_t)

                # x fp32 -> bf16: first chunks on DVE here; the rest are
                # converted inside the it=0 loop (ensure_chunk) so the DVE
                # queue never head-of-line blocks on a late DMA.
                for k in range(n_xsync):
                    for cc in range(2):
                        sl = slice(k * XCHUNK, (k + 1) * XCHUNK)
                        nc.scalar.copy(x_sb[cc][:, sl], x_stage[cc][k])
                n_pro = n_xsync

            # ---------------- attention + output projection ----------------
            with tc.tile_pool(name="sc_ps", bufs=2, space="PSUM") as sc_ps, \
                 tc.tile_pool(name="u_ps", bufs=2, space="PSUM") as u_ps, \
                 tc.tile_pool(name="qk_ps", bufs=2, space="PSUM") as qk_ps, \
                 tc.tile_pool(name="pt_sb", bufs=4) as pt_sb, \
                 tc.tile_pool(name="o_sb", bufs=2) as o_sb:

                converted = [False] * n_xchunks

                def ensure_chunk(k):
                    if converted[k]:
                        return
                    converted[k] = True
                    for cc in range(2):
                        sl = slice(k * XCHUNK, (k + 1) * XCHUNK)
                        nc.vector.tensor_copy(x_sb[cc][:, sl], x_stage[cc][k])

                for k in range(n_pro):
                    converted[k] = True

                def emit_qkv(nt):
                    # qkv projection for one n-tile
                    ensure_chunk(nt * NT // XCHUNK)
                    ensure_chunk(((nt + 1) * NT - 1) // XCHUNK)
                    sl = slice(nt * NT, (nt + 1) * NT)
                    ps_q = qk_ps.tile([128, NT], f32, tag="qk",
                                      name=f"psq_{nt}")
                    for cc in range(2):
                        nc.tensor.matmul(ps_q, q4T[cc], x_sb[cc][:, sl],
                                         start=(cc == 0), stop=(cc == 1))
                    nc.vector.tensor_scalar_add(q4[:, sl], ps_q, bq4)
                    ps_k = qk_ps.tile([128, NT], f32, tag="qk",
                                      name=f"psk_{nt}")
                    for cc in range(2):
                        nc.tensor.matmul(ps_k, k4T[cc], x_sb[cc][:, sl],
                                         start=(cc == 0), stop=(cc == 1))
                    nc.vector.tensor_copy(k4[:, sl], ps_k)
                    ps_v = qk_ps.tile([D, NT], f32, tag="qk",
                                      name=f"psv_{nt}")
                    for cc in range(2):
                        nc.tensor.matmul(ps_v, vT_w[cc], x_sb[cc][:, sl],
                                         start=(cc == 0), stop=(cc == 1))
                    nc.vector.tensor_scalar_add(v_sb[0:D, sl], ps_v, bv)
                    # V^T for the 4 j-tiles of this n-tile, one contiguous copy
                    ps_vt = qk_ps.tile([128, 4 * VTS], bf16, tag="qk",
                                       name=f"psvt_{nt}")
                    for jj in range(4):
                        j = 4 * nt + jj
                        nc.tensor.transpose(
                            ps_vt[:, jj * VTS:(jj + 1) * VTS],
                            v_sb[:, j * JT:(j + 1) * JT],
                            ident_bf[0:VTS, 0:VTS])
                    nc.vector.tensor_copy(
                        vt_all[:, 4 * nt * VTS:(4 * nt + 4) * VTS],
                        ps_vt)

                def emit_scores(it, g):
                    # 4 j-tiles concurrently in the 4 PE row groups
                    isl = slice(it * NT, (it + 1) * NT)
                    S1 = sc_ps.tile([128, 2 * NT], f32, tag="sc",
                                    name=f"S1_{it}_{g}")
                    S2 = sc_ps.tile([128, 2 * NT], f32, tag="sc",
                                    name=f"S2_{it}_{g}")
                    for r, (Sx, off) in enumerate(
                            ((S1, 0), (S1, NT), (S2, 0), (S2, NT))):
                        j = 4 * g + r
                        nc.tensor.matmul(
                            Sx[:, off:off + NT],
                            k4[D * r:D * (r + 1), j * JT:(j + 1) * JT],
                            q4[D * r:D * (r + 1), isl],
                            start=True, stop=True, tile_position=(32 * r, 0),
                        )
                    PT1 = pt_sb.tile([128, 2 * NT], bf16, tag="pt",
                                     name=f"PT1_{it}_{g}")
                    nc.scalar.activation(PT1, S1, AF.Exp, scale=SCALE)
                    PT2 = pt_sb.tile([128, 2 * NT], bf16, tag="pt",
                                     name=f"PT2_{it}_{g}")
                    nc.scalar.activation(PT2, S2, AF.Exp, scale=SCALE)
                    return PT1, PT2

                def emit_u(U, g, PT1, PT2):
                    st, sp = (g == 0), (g == n_quads - 1)
                    for idx, (PTx, off) in enumerate(
                            ((PT1, 0), (PT1, NT), (PT2, 0), (PT2, NT))):
                        j = 4 * g + idx
                        lo = 0 if idx % 2 == 0 else 64
                        nc.tensor.matmul(
                            U[lo:lo + D + 1, :],
                            vt_all[:, j * VTS:j * VTS + D + 1],
                            PTx[:, off:off + NT],
                            start=(st and idx < 2), stop=(sp and idx >= 2),
                            tile_position=(0, lo),
                            skip_group_check=True,
                        )

                def emit_norm(it, U):
                    # deferred normalization: combine the two column-group
                    # halves (bf16) and kick off the reciprocal+broadcast;
                    # the projection runs on the UNNORMALIZED sums and gets
                    # scaled afterwards (columnwise 1/denom commutes with Wo).
                    usum_b = o_sb.tile([D + 1, NT], f32, tag="usum_b",
                                       name=f"usum_b_{it}")
                    nc.vector.tensor_copy(usum_b, U[64:64 + D + 1, :])
                    usum = o_sb.tile([D + 1, NT], bf16, tag="usum",
                                     name=f"usum_{it}")
                    nc.vector.tensor_add(usum, U[0:D + 1, :], usum_b)
                    rcp = o_sb.tile([1, NT], f32, tag="rcp", name=f"rcp_{it}")
                    nc.vector.reciprocal(rcp, usum[D:D + 1, :])
                    rb = o_sb.tile([128, NT], f32, tag="rb", name=f"rb_{it}")
                    nc.gpsimd.partition_broadcast(rb, rcp)
                    return usum, rb

                def emit_proj_mm(pit, usum):
                    ps_os = []
                    for hh in range(2):
                        ps_o = qk_ps.tile([128, NT], f32, tag="qk",
                                          name=f"ps_o_{pit}_{hh}")
                        nc.tensor.matmul(ps_o, woT_aug[hh], usum,
                                         start=True, stop=True)
                        ps_os.append(ps_o)
                    return ps_os

                def emit_proj_fin(pit, ps_os, rb):
                    psl = slice(pit * NT, (pit + 1) * NT)
                    for hh in range(2):
                        o_out = o_sb.tile([128, NT], f32, tag="o_out",
                                          name=f"o_out_{pit}_{hh}")
                        nc.vector.tensor_mul(o_out, ps_os[hh], rb)
                        nc.sync.dma_start(
                            out=out[128 * hh:128 * (hh + 1), psl], in_=o_out
                        )

                # i-tile 0 fused with the qkv/V^T build: quad g only needs
                # k/v/VT of n-tile g (and q of n-tile 0).
                assert n_quads == n_itiles
                # Software-pipelined: the last U matmuls of i-tile it-1 and
                # its norm/proj are emitted inside i-tile it's block, AFTER
                # scores(it, 0) -- so the in-order PE queue fills the next
                # S buffer during the last exp instead of idling.
                U0 = u_ps.tile([97, NT], f32, tag="u", name="U_0")
                emit_qkv(0)
                prev = emit_scores(0, 0)
                for g in range(1, n_quads):
                    emit_qkv(g)
                    cur = emit_scores(0, g)
                    emit_u(U0, g - 1, *prev)
                    prev = cur
                carry = (U0, 0, prev)

                g_mm = min(2, n_quads - 1)
                g_fin = min(5, n_quads - 1)
                for it in range(1, n_itiles):
                    U = u_ps.tile([97, NT], f32, tag="u", name=f"U_{it}")
                    prev = emit_scores(it, 0)
                    Uc, itc, sc_prev = carry
                    emit_u(Uc, n_quads - 1, *sc_prev)
                    usum_c, rb_c = emit_norm(itc, Uc)
                    pmm = None
                    for g in range(1, n_quads):
                        cur = emit_scores(it, g)
                        emit_u(U, g - 1, *prev)
                        prev = cur
                        if g == g_mm:
                            pmm = emit_proj_mm(itc, usum_c)
                        if g == g_fin and pmm is not None:
                            emit_proj_fin(itc, pmm, rb_c)
                    if pmm is None:
                        pmm = emit_proj_mm(itc, usum_c)
                        emit_proj_fin(itc, pmm, rb_c)
                    carry = (U, it, prev)

                Uc, itc, sc_prev = carry
                emit_u(Uc, n_quads - 1, *sc_prev)
                usum_c, rb_c = emit_norm(itc, Uc)
                emit_proj_fin(itc, emit_proj_mm(itc, usum_c), rb_c)
    nc.compile()
    return nc


@functools.lru_cache(maxsize=2)
def _get_nc(n_pos=N_FULL):
    return _build_program(n_pos)


def _make_in_maps(x, time_emb, w_mlp, b_mlp, w_qkv, w_out, b_out, n_pos=N_FULL):
    x = np.ascontiguousarray(np.asarray(x, dtype=np.float32))
    time_emb = np.ascontiguousarray(np.asarray(time_emb, dtype=np.float32))
    w_mlp = np.ascontiguousarray(np.asarray(w_mlp, dtype=np.float32))
    b_mlp = np.ascontiguousarray(np.asarray(b_mlp, dtype=np.float32))
    w_qkv = np.ascontiguousarray(np.asarray(w_qkv, dtype=np.float32))
    w_out = np.ascontiguousarray(np.asarray(w_out, dtype=np.float32))
    b_out = np.ascontiguousarray(np.asarray(b_out, dtype=np.float32))

    b = x.shape[0]
    hid = HEADS * D
    in_maps = []
    for core in range(N_CORES):
        bb, hh = core // HEADS, core % HEADS
        in_maps.append({
            "xb": np.ascontiguousarray(
                x[bb].reshape(C, -1)[:, :n_pos]),
            "te": time_emb[bb],
            "w_mlp": w_mlp,
            "b_mlp": b_mlp,
            "wqkv": np.ascontiguousarray(np.concatenate([
                w_qkv[D * hh:D * (hh + 1), :],
                w_qkv[hid + D * hh:hid + D * (hh + 1), :],
                w_qkv[2 * hid + D * hh:2 * hid + D * (hh + 1), :]], axis=0)),
            "wo": np.ascontiguousarray(w_out[:, D * hh:D * (hh + 1)]),
            "bo": b_out if hh == 0 else np.zeros_like(b_out),
        })
    return in_maps


def _install_ntff_hook():
    """Register the axon NTFF profile hook (the agent image's antenv lacks
    axon_hooks; replicate trn_boot's ctypes shim so trace=True works)."""
    import types
    import contextlib
    import ctypes

    try:
        from antenv.axon_hooks import get_axon_ntff_profile_hook  # noqa: F401
        return
    except ImportError:
        pass
    so_path = "/opt/axon/libaxon_pjrt.so"
    try:
        lib = ctypes.CDLL(so_path)
    except OSError:
        return
    if not hasattr(lib, "axon_start_nrt_profile"):
        return
    lib.axon_start_nrt_profile.argtypes = [
        ctypes.POINTER(ctypes.c_int64), ctypes.c_size_t]
    lib.axon_start_nrt_profile.restype = ctypes.c_int64
    lib.axon_stop_nrt_profile.argtypes = [ctypes.c_char_p]
    lib.axon_stop_nrt_profile.restype = ctypes.c_int64

    @contextlib.contextmanager
    def _hook(output_dir, device_ids):
        import jax
        jax.devices()
        if device_ids:
            ids = (ctypes.c_int64 * len(device_ids))(*device_ids)
            rc = lib.axon_start_nrt_profile(ids, len(device_ids))
        else:
            rc = lib.axon_start_nrt_profile(None, 0)
        if rc != 0:
            raise RuntimeError(f"axon_start_nrt_profile rc={rc}")
        try:
            yield
        finally:
            n = lib.axon_stop_nrt_profile(str(output_dir).encode())
            print(f"profile: {n} file(s) written to {output_dir}",
                  file=sys.stderr)

    import antenv
    mod = types.ModuleType("antenv.axon_hooks")
    mod.get_axon_ntff_profile_hook = lambda: _hook
    mod.set_axon_ntff_profile_hook = lambda h: None
    sys.modules["antenv.axon_hooks"] = mod
    antenv.axon_hooks = mod


def _run(inputs, trace=False, n_pos=N_FULL):
    from concourse.bass_utils import run_bass_kernel_spmd

    if trace:
        _install_ntff_hook()
    nc = _get_nc(n_pos)
    in_maps = _make_in_maps(**inputs, n_pos=n_pos)
    res = run_bass_kernel_spmd(
        nc, in_maps, core_ids=list(range(N_CORES)), trace=trace
    )
    return res


def _assemble(results, x_shape):
    b, c, h, w = x_shape
    out = np.zeros((b, c, h * w), dtype=np.float32)
    for core in range(N_CORES):
        bb = core // HEADS
        out[bb] += results[core]["out"]
    return out.reshape(b, c, h, w)


def kernel(x, time_emb, w_mlp, b_mlp, w_qkv, w_out, b_out):
    res = _run(dict(
        x=x, time_emb=time_emb, w_mlp=w_mlp, b_mlp=b_mlp,
        w_qkv=w_qkv, w_out=w_out, b_out=b_out,
    ))
    return _assemble(res.results, np.asarray(x).shape)
